# revision 15
# baseline (speedup 1.0000x reference)
"""256-point FFT (real/imag channels) as a DFT matmul on Trainium2.

Contract: kernel(x) takes the FULL input x [131072, 2, 256] float32 and
returns the FULL output [131072, 2, 256] float32, computing, per batch row,
the 256-point complex FFT of (x[b,0,:] + i*x[b,1,:]) -> [real; imag].

Strategy (pure data parallel over 8 NeuronCores, 16384 rows/core):
  - Flatten each row to v[512] = [re(256), im(256)].  The FFT is the linear
    map y = v @ W with W[512,512] built from cos/sin of the DFT twiddles.
  - Per core, stream 2048-row super-chunks (4 MiB DMAs; 16 consecutive rows
    per partition so each partition's slice is one contiguous 32 KiB run,
    cast f32->bf16 in the SWDGE datapath during the load).  For each 128-row
    sub-chunk: transpose each [128b x 128n] block on TensorE (transpose
    mode, via identity) into PSUM, copy back to SBUF (VectorE), then 4
    accumulating matmuls against W (bf16 data/weights, fp32 PSUM
    accumulation) produce [128b x 512out] batch-major, which ScalarE copies
    to SBUF and HWDGE DMAs out.
  - bf16 inputs + fp32 accumulate gives ~2.1e-3 relative error vs the fp32
    reference.  DMA is the roofline: 64 MiB/core of HBM traffic at the
    ~358 GB/s per-core HBM limit = ~186 us; measured ~188-194 us.
"""

import numpy as np

B_TOTAL = 131072
N_CORES = 8
B_CORE = B_TOTAL // N_CORES  # 16384
NFFT = 256
V = 2 * NFFT  # 512, flattened row length
P = 128  # partitions

_cache = {}


def _dft_matrix_f64():
    """W[n, m] such that out[b, m] = sum_n v[b, n] * W[n, m]."""
    k = np.arange(NFFT, dtype=np.float64)
    theta = -2.0 * np.pi * np.outer(k, k) / NFFT  # [k, n]
    c = np.cos(theta).T  # [n, k]
    s = np.sin(theta).T  # [n, k]
    w = np.zeros((V, V), np.float64)
    w[:NFFT, :NFFT] = c
    w[NFFT:, :NFFT] = -s
    w[:NFFT, NFFT:] = s
    w[NFFT:, NFFT:] = c
    return w


def _build(b_core, super_rows, variant="bf16"):
    """Build + compile the per-core Bass program. Returns nc.

    variant: "bf16" (cast input to bf16 during load; ~2e-3 rel err) or
    "f32r" (fp32-width data, PE fp32r decomposition; higher precision).
    """
    import concourse.bass as bass
    import concourse.tile as tile
    from concourse import bacc, mybir

    n_super = b_core // super_rows
    r_sub = super_rows // P  # 128-row sub-chunks per DMA super-chunk
    f32 = mybir.dt.float32
    cdt = mybir.dt.bfloat16 if variant == "bf16" else mybir.dt.float32r

    nc = bacc.Bacc(
        "TRN2",
        target_bir_lowering=False,
        debug=False,
        num_devices=N_CORES,
    )
    x_dt = f32 if variant == "bf16" else mybir.dt.float32r
    x_d = nc.dram_tensor("x_in", [b_core, V], x_dt, kind="ExternalInput")
    w_d = nc.dram_tensor("w_in", [V, V], cdt, kind="ExternalInput")
    id_d = nc.dram_tensor("id_in", [P, P], cdt, kind="ExternalInput")
    y_d = nc.dram_tensor("y_out", [b_core, V], f32, kind="ExternalOutput")

    with tile.TileContext(nc) as tc:
        with (
            tc.tile_pool(name="const", bufs=1) as cpool,
            tc.tile_pool(name="xin", bufs=3) as xpool,
            tc.tile_pool(name="xt", bufs=4) as xtpool,
            tc.tile_pool(name="yout", bufs=3) as ypool,
            tc.tile_pool(name="psumT", bufs=3, space="PSUM") as ptpool,
            tc.tile_pool(name="psumO", bufs=3, space="PSUM") as popool,
        ):
            w_sb = cpool.tile([P, 4, V], cdt)
            nc.sync.dma_start(w_sb[:], w_d.ap().rearrange("(j p) m -> p j m", p=P))
            id_sb = cpool.tile([P, P], cdt)
            nc.sync.dma_start(id_sb[:], id_d.ap())

            for t in range(n_super):
                # Map r_sub *consecutive* DRAM rows to each partition so every
                # partition's slice of the transfer is one contiguous run
                # (vs 2 KiB descriptors with row-round-robin layout).
                xin = xpool.tile([P, r_sub, V], cdt)
                load_eng = nc.gpsimd if variant == "bf16" else nc.sync
                load_eng.dma_start(
                    xin[:],
                    x_d.ap()[t * super_rows : (t + 1) * super_rows, :].rearrange(
                        "(p r) e -> p r e", p=P
                    ),
                )
                yout = ypool.tile([P, r_sub, V], f32)
                for r in range(r_sub):
                    psum_t = ptpool.tile([P, V], cdt)
                    for j in range(4):
                        nc.tensor.transpose(
                            psum_t[:, j * P : (j + 1) * P],
                            xin[:, r, j * P : (j + 1) * P],
                            id_sb[:],
                        )
                    xt = xtpool.tile([P, V], cdt)
                    nc.vector.tensor_copy(xt[:], psum_t[:])
                    psum_o = popool.tile([P, V], f32)
                    for j in range(4):
                        nc.tensor.matmul(
                            psum_o[:],
                            xt[:, j * P : (j + 1) * P],
                            w_sb[:, j, :],
                            start=(j == 0),
                            stop=(j == 3),
                        )
                    nc.scalar.copy(yout[:, r, :], psum_o[:])
                nc.sync.dma_start(
                    y_d.ap()[t * super_rows : (t + 1) * super_rows, :].rearrange(
                        "(p r) e -> p r e", p=P
                    ),
                    yout[:],
                )

    nc.compile()
    return nc


VARIANT = "bf16"
SUPER_ROWS = 2048


def _get_program(variant):
    key = ("prog", B_CORE, SUPER_ROWS, variant)
    if key not in _cache:
        _cache[key] = _build(B_CORE, SUPER_ROWS, variant)
    return _cache[key]


def _input_consts(variant):
    import ml_dtypes

    key = ("consts", variant)
    if key not in _cache:
        wdt = ml_dtypes.bfloat16 if variant == "bf16" else np.float32
        w = _dft_matrix_f64().astype(wdt)
        ident = np.eye(P, dtype=wdt)
        _cache[key] = (w, ident)
    return _cache[key]


def _run(x, trace=False, trace_cores=None, variant=None):
    """x: [B_TOTAL, 2, 256] f32 -> (out [B_TOTAL, 2, 256] f32, results obj)."""
    from concourse import bass_utils

    variant = variant or VARIANT
    x = np.ascontiguousarray(np.asarray(x, dtype=np.float32)).reshape(B_TOTAL, V)
    w, ident = _input_consts(variant)
    nc = _get_program(variant)
    in_maps = [
        {
            "x_in": x[c * B_CORE : (c + 1) * B_CORE],
            "w_in": w,
            "id_in": ident,
        }
        for c in range(N_CORES)
    ]
    res = bass_utils.run_bass_kernel_spmd(
        nc,
        in_maps,
        core_ids=list(range(N_CORES)),
        trace=trace,
        trace_cores=trace_cores,
    )
    out = np.concatenate([res.results[c]["y_out"] for c in range(N_CORES)], axis=0)
    return out.reshape(B_TOTAL, 2, NFFT).astype(np.float32, copy=False), res


def kernel(x):
    out, _ = _run(x, trace=False)
    return out


# revision 28
# speedup vs baseline: 1.1954x; 1.1954x over previous
"""256-point FFT (real/imag channels) as a DFT matmul on Trainium2.

Contract: kernel(x) takes the FULL input x [131072, 2, 256] float32 and
returns the FULL output [131072, 2, 256] float32, computing, per batch row,
the 256-point complex FFT of (x[b,0,:] + i*x[b,1,:]) -> [real; imag].

Strategy (pure data parallel over 8 NeuronCores, 16384 rows/core):
  - Flatten each row to v[512] = [re(256), im(256)].  The FFT is a linear
    map; it is evaluated split-radix style as two 128-point DFT matmuls
    (even/odd input samples, the odd-side twiddle folded into its matrix)
    followed by a VectorE add/sub butterfly: X[k] = E[k] + O'[k],
    X[k+128] = E[k] - O'[k].  This halves TensorE streaming cycles vs the
    full 512x512 DFT matmul.
  - Per core, stream 2048-row super-chunks (4 MiB DMAs; 16 consecutive
    rows per partition so each partition's slice is one contiguous run,
    cast f32->bf16 in the SWDGE datapath during the load).  Per 128-row
    sub-chunk: TensorE transposes four de-interleaved [128b x 128m] blocks
    (even-re/odd-re/even-im/odd-im) into PSUM, VectorE copies them back to
    SBUF, 2+2 accumulating bf16 matmuls (fp32 PSUM) produce [E_re|E_im]
    and [O_re|O_im] batch-major, ScalarE copies PSUM->SBUF casting to
    bf16, VectorE butterflies into the output tile, HWDGE stores bf16
    (host upcasts to f32 — halves write traffic; HBM traffic is 48
    MiB/core total).
  - Relative error vs the fp32 reference: ~3.2e-3 (resid_var ~1.1e-5).
    Measured ~159-168 us/core vs a ~140 us HBM floor; DMA/PE/DVE all
    within ~15% of each other at the end.
"""

import numpy as np

B_TOTAL = 131072
N_CORES = 8
B_CORE = B_TOTAL // N_CORES  # 16384
NFFT = 256
V = 2 * NFFT  # 512, flattened row length
P = 128  # partitions

_cache = {}


def _dft_matrix_f64():
    """W[n, m] such that out[b, m] = sum_n v[b, n] * W[n, m]."""
    k = np.arange(NFFT, dtype=np.float64)
    theta = -2.0 * np.pi * np.outer(k, k) / NFFT  # [k, n]
    c = np.cos(theta).T  # [n, k]
    s = np.sin(theta).T  # [n, k]
    w = np.zeros((V, V), np.float64)
    w[:NFFT, :NFFT] = c
    w[NFFT:, :NFFT] = -s
    w[:NFFT, NFFT:] = s
    w[NFFT:, NFFT:] = c
    return w


def _sr_matrices_f64():
    """Split-radix weights: two 128-point DFTs with the odd-side twiddle
    folded in.  Returns [512, 256]: stacked [WEr; WOr; WEi; WOi] blocks,
    block j multiplying transposed-data block j (even-re, odd-re, even-im,
    odd-im).  Output cols = [E_re|E_im] (even blocks) / [O_re|O_im] (odd)."""
    k = np.arange(P, dtype=np.float64)
    m = np.arange(P, dtype=np.float64)
    th_e = -2.0 * np.pi * np.outer(k, 2 * m) / NFFT  # [k, m]
    th_o = -2.0 * np.pi * np.outer(k, 2 * m + 1) / NFFT
    w = np.zeros((4, P, 2 * P), np.float64)
    for j, th, imag_src in ((0, th_e, False), (1, th_o, False), (2, th_e, True), (3, th_o, True)):
        c, s = np.cos(th).T, np.sin(th).T  # [m, k]
        w[j, :, :P] = -s if imag_src else c
        w[j, :, P:] = c if imag_src else s
    return w.reshape(4 * P, 2 * P)


def _build(b_core, super_rows, variant="bf16"):
    """Build + compile the per-core Bass program. Returns nc.

    variant: "bf16" (cast input to bf16 during load; ~2e-3 rel err),
    "sr" (bf16 + split-radix: two 128-DFT matmuls + DVE butterfly), or
    "f32r" (fp32-width data, PE fp32r decomposition; higher precision).
    """
    import concourse.bass as bass
    import concourse.tile as tile
    from concourse import bacc, mybir

    n_super = b_core // super_rows
    r_sub = super_rows // P  # 128-row sub-chunks per DMA super-chunk
    f32 = mybir.dt.float32
    sr = variant == "sr"
    cdt = mybir.dt.float32r if variant == "f32r" else mybir.dt.bfloat16
    n_wcol = 2 * P if sr else V

    nc = bacc.Bacc(
        "TRN2",
        target_bir_lowering=False,
        debug=False,
        num_devices=N_CORES,
    )
    x_dt = mybir.dt.float32r if variant == "f32r" else f32
    x_d = nc.dram_tensor("x_in", [b_core, V], x_dt, kind="ExternalInput")
    w_d = nc.dram_tensor("w_in", [V, n_wcol], cdt, kind="ExternalInput")
    id_d = nc.dram_tensor("id_in", [P, P], cdt, kind="ExternalInput")
    # bf16/sr variants also *store* bf16 (host upcasts to f32): halves the
    # HBM write traffic (64 -> 48 MiB/core total), which is the roofline.
    y_dt = f32 if variant == "f32r" else mybir.dt.bfloat16
    y_d = nc.dram_tensor("y_out", [b_core, V], y_dt, kind="ExternalOutput")

    with tile.TileContext(nc) as tc:
        with (
            tc.tile_pool(name="const", bufs=1) as cpool,
            tc.tile_pool(name="xin", bufs=3) as xpool,
            tc.tile_pool(name="xt", bufs=4) as xtpool,
            tc.tile_pool(name="yout", bufs=3) as ypool,
            tc.tile_pool(name="psumT", bufs=3, space="PSUM") as ptpool,
            tc.tile_pool(name="psumO", bufs=3, space="PSUM") as popool,
        ):
            w_sb = cpool.tile([P, 4, n_wcol], cdt)
            nc.sync.dma_start(w_sb[:], w_d.ap().rearrange("(j p) m -> p j m", p=P))
            id_sb = cpool.tile([P, P], cdt)
            nc.sync.dma_start(id_sb[:], id_d.ap())

            for t in range(n_super):
                # Map r_sub *consecutive* DRAM rows to each partition so every
                # partition's slice of the transfer is one contiguous run
                # (vs 2 KiB descriptors with row-round-robin layout).
                xin = xpool.tile([P, r_sub, V], cdt)
                load_eng = nc.sync if variant == "f32r" else nc.gpsimd
                load_eng.dma_start(
                    xin[:],
                    x_d.ap()[t * super_rows : (t + 1) * super_rows, :].rearrange(
                        "(p r) e -> p r e", p=P
                    ),
                )
                yout = ypool.tile([P, r_sub, V], y_dt)
                for r in range(r_sub):
                    psum_t = ptpool.tile([P, V], cdt)
                    if sr:
                        # Transpose de-interleaved blocks: (q=parity, h=re/im)
                        # -> block j in [even-re, odd-re, even-im, odd-im].
                        xv = xin[:, r, :].rearrange("p (h m q) -> p q h m", q=2, h=2)
                        for j, (q, h) in enumerate(((0, 0), (1, 0), (0, 1), (1, 1))):
                            nc.tensor.transpose(
                                psum_t[:, j * P : (j + 1) * P],
                                xv[:, q, h, :],
                                id_sb[:],
                            )
                    else:
                        for j in range(4):
                            nc.tensor.transpose(
                                psum_t[:, j * P : (j + 1) * P],
                                xin[:, r, j * P : (j + 1) * P],
                                id_sb[:],
                            )
                    xt = xtpool.tile([P, V], cdt)
                    nc.vector.tensor_copy(xt[:], psum_t[:])
                    psum_o = popool.tile([P, V], f32)
                    if sr:
                        # E = DFT128(even) into cols 0:256, O' = twiddled
                        # DFT128(odd) into cols 256:512.
                        nc.tensor.matmul(psum_o[:, 0:256], xt[:, 0:P], w_sb[:, 0, :], start=True, stop=False)
                        nc.tensor.matmul(psum_o[:, 0:256], xt[:, 2 * P : 3 * P], w_sb[:, 2, :], start=False, stop=True)
                        nc.tensor.matmul(psum_o[:, 256:512], xt[:, P : 2 * P], w_sb[:, 1, :], start=True, stop=False)
                        nc.tensor.matmul(psum_o[:, 256:512], xt[:, 3 * P : 4 * P], w_sb[:, 3, :], start=False, stop=True)
                        eo = xtpool.tile([P, V], cdt, tag="eo")
                        nc.scalar.copy(eo[:], psum_o[:])
                        er, ei = eo[:, 0:P], eo[:, P : 2 * P]
                        orr, oi = eo[:, 2 * P : 3 * P], eo[:, 3 * P : 4 * P]
                        yr = yout[:, r, :]
                        nc.vector.tensor_add(yr[:, 0:P], er, orr)
                        nc.vector.tensor_sub(yr[:, P : 2 * P], er, orr)
                        nc.vector.tensor_add(yr[:, 2 * P : 3 * P], ei, oi)
                        nc.vector.tensor_sub(yr[:, 3 * P : 4 * P], ei, oi)
                    else:
                        for j in range(4):
                            nc.tensor.matmul(
                                psum_o[:],
                                xt[:, j * P : (j + 1) * P],
                                w_sb[:, j, :],
                                start=(j == 0),
                                stop=(j == 3),
                            )
                        nc.scalar.copy(yout[:, r, :], psum_o[:])
                nc.sync.dma_start(
                    y_d.ap()[t * super_rows : (t + 1) * super_rows, :].rearrange(
                        "(p r) e -> p r e", p=P
                    ),
                    yout[:],
                )

    nc.compile()
    return nc


VARIANT = "sr"
SUPER_ROWS = 2048


def _get_program(variant):
    key = ("prog", B_CORE, SUPER_ROWS, variant)
    if key not in _cache:
        _cache[key] = _build(B_CORE, SUPER_ROWS, variant)
    return _cache[key]


def _input_consts(variant):
    import ml_dtypes

    key = ("consts", variant)
    if key not in _cache:
        wdt = np.float32 if variant == "f32r" else ml_dtypes.bfloat16
        w64 = _sr_matrices_f64() if variant == "sr" else _dft_matrix_f64()
        w = w64.astype(wdt)
        ident = np.eye(P, dtype=wdt)
        _cache[key] = (w, ident)
    return _cache[key]


def _run(x, trace=False, trace_cores=None, variant=None):
    """x: [B_TOTAL, 2, 256] f32 -> (out [B_TOTAL, 2, 256] f32, results obj)."""
    from concourse import bass_utils

    variant = variant or VARIANT
    x = np.ascontiguousarray(np.asarray(x, dtype=np.float32)).reshape(B_TOTAL, V)
    w, ident = _input_consts(variant)
    nc = _get_program(variant)
    in_maps = [
        {
            "x_in": x[c * B_CORE : (c + 1) * B_CORE],
            "w_in": w,
            "id_in": ident,
        }
        for c in range(N_CORES)
    ]
    res = bass_utils.run_bass_kernel_spmd(
        nc,
        in_maps,
        core_ids=list(range(N_CORES)),
        trace=trace,
        trace_cores=trace_cores,
    )
    out = np.concatenate(
        [np.asarray(res.results[c]["y_out"], dtype=np.float32) for c in range(N_CORES)],
        axis=0,
    )
    return out.reshape(B_TOTAL, 2, NFFT), res


def kernel(x):
    out, _ = _run(x, trace=False)
    return out
